# revision 2
# baseline (speedup 1.0000x reference)
"""Trainium kernel for nn_BasicVSR_LFN: upflow + backwarp + 7x7 correlation + 4 convs.

Strategy (per sharding hint): pure data-parallel over the batch dim. B=8 samples
-> one sample per NeuronCore across the 8 cores, weights replicated. All
arithmetic runs on the NeuronCores via the axon PJRT backend; the host only
shards inputs / gathers outputs.

All convolutions are expressed as shifted-slice matmul sums (dot_general) and
the transposed conv as dilate+shifted-adds, because this container's
neuronx-cc build lacks `private_nkl` and its TransformConvOp path ICEs on
conv_general_dilated; dot_general lowers through the robust matmul path onto
the tensor engine.
"""
import numpy as np
import jax
import jax.numpy as jnp

MD = 3  # max displacement -> 7x7 = 49 correlation channels

# hardcoded problem shapes (self-contained; no reading of spec/reference)
B, C, H, W = 8, 96, 96, 160
N_CORES = 8


def _lrelu(x):
    return jnp.where(x >= 0, x, 0.1 * x)


def _conv_mm(x, w, b, p):
    """x [Cin,H,W], w [O,Cin,k,k], pad p -> [O,H,W] via k*k shifted matmuls."""
    cin, h, wd = x.shape
    o, _, k, _ = w.shape
    xp = jnp.pad(x, ((0, 0), (p, p), (p, p)))
    acc = jnp.zeros((o, h * wd), x.dtype)
    for ky in range(k):
        for kx in range(k):
            xs = xp[:, ky:ky + h, kx:kx + wd].reshape(cin, h * wd)
            acc = acc + jnp.dot(w[:, :, ky, kx], xs)
    return acc.reshape(o, h, wd) + b[:, None, None]


def _upflow(flow, w):
    """grouped ConvTranspose2d(2,2,k=4,s=2,p=1,groups=2) as dilate+shift-adds.
    flow [2,h,w] -> [2,2h,2w]"""
    wf = jnp.flip(w, axis=(2, 3))  # [2,1,4,4]
    g, hh, ww = flow.shape
    D = jnp.zeros((g, 2 * hh + 3, 2 * ww + 3), flow.dtype)
    D = D.at[:, 2:2 + 2 * hh:2, 2:2 + 2 * ww:2].set(flow)
    out = jnp.zeros((g, 2 * hh, 2 * ww), flow.dtype)
    for a in range(4):
        for bb in range(4):
            out = out + D[:, a:a + 2 * hh, bb:bb + 2 * ww] * \
                wf[:, 0, a, bb][:, None, None]
    return out


def _backwarp(x, flow):
    """x [C,H,W], flow [2,H,W]; grid_sample bilinear, zeros padding,
    align_corners=False."""
    c, h, w = x.shape
    gx = jnp.linspace(-1.0 + 1.0 / w, 1.0 - 1.0 / w, w, dtype=x.dtype)
    gy = jnp.linspace(-1.0 + 1.0 / h, 1.0 - 1.0 / h, h, dtype=x.dtype)
    nx = gx[None, :] + flow[0] / ((w - 1.0) / 2.0)   # [H,W]
    ny = gy[:, None] + flow[1] / ((h - 1.0) / 2.0)
    px = (nx + 1.0) * w / 2.0 - 0.5
    py = (ny + 1.0) * h / 2.0 - 0.5
    x0 = jnp.floor(px)
    y0 = jnp.floor(py)
    wx1 = px - x0
    wx0 = 1.0 - wx1
    wy1 = py - y0
    wy0 = 1.0 - wy1
    x0i = x0.astype(jnp.int32)
    y0i = y0.astype(jnp.int32)
    x1i = x0i + 1
    y1i = y0i + 1
    imgf = x.reshape(c, h * w)

    def samp(iy, ix):
        valid = ((iy >= 0) & (iy < h) & (ix >= 0) & (ix < w)).astype(x.dtype)
        idx = (jnp.clip(iy, 0, h - 1) * w + jnp.clip(ix, 0, w - 1)).reshape(-1)
        v = jnp.take(imgf, idx, axis=1)
        return v.reshape(c, h, w) * valid[None]

    return (samp(y0i, x0i) * (wy0 * wx0)[None]
            + samp(y0i, x1i) * (wy0 * wx1)[None]
            + samp(y1i, x0i) * (wy1 * wx0)[None]
            + samp(y1i, x1i) * (wy1 * wx1)[None])


def _correlation(a, bwp):
    """a,bwp [C,H,W] -> [49,H,W]: mean over C of shifted products."""
    c, h, w = a.shape
    bp = jnp.pad(bwp, ((0, 0), (MD, MD), (MD, MD)))
    outs = []
    for dy in range(2 * MD + 1):
        for dx in range(2 * MD + 1):
            outs.append(jnp.mean(a * bp[:, dy:dy + h, dx:dx + w], axis=0))
    return jnp.stack(outs, axis=0)


def _forward(feat_one, feat_two, flow_prev, up_w,
             w1, b1, w2, b2, w3, b3, w4, b4):
    # per-sample forward: feat [1,C,H,W] squeezed to [C,H,W]
    f1 = feat_one[0]
    f2 = feat_two[0]
    fp = flow_prev[0]
    FLT_BACKWARP = 2.5
    flow = _upflow(fp, up_w)                       # [2,H,W]
    warped = _backwarp(f2, flow * FLT_BACKWARP)    # [C,H,W]
    corr = _lrelu(_correlation(f1, warped))        # [49,H,W]
    h1 = _lrelu(_conv_mm(corr, w1, b1, 1))
    h2 = _lrelu(_conv_mm(h1, w2, b2, 1))
    h3 = _lrelu(_conv_mm(h2, w3, b3, 1))
    res = _conv_mm(h3, w4, b4, 2)
    return (flow + res)[None]                      # [1,2,H,W]


_PFN = None


def _get_pfn():
    global _PFN
    if _PFN is None:
        devs = jax.devices()[:N_CORES]
        _PFN = jax.pmap(
            _forward,
            in_axes=(0, 0, 0) + (None,) * 9,
            devices=devs,
        )
    return _PFN


def kernel(feat_one, feat_two, flow_prev, up_w,
           w1, b1, w2, b2, w3, b3, w4, b4):
    pfn = _get_pfn()
    f1 = np.asarray(feat_one, np.float32).reshape(N_CORES, 1, C, H, W)
    f2 = np.asarray(feat_two, np.float32).reshape(N_CORES, 1, C, H, W)
    fp = np.asarray(flow_prev, np.float32).reshape(N_CORES, 1, 2, H // 2, W // 2)
    out = pfn(f1, f2, fp,
              jnp.asarray(up_w), jnp.asarray(w1), jnp.asarray(b1),
              jnp.asarray(w2), jnp.asarray(b2), jnp.asarray(w3),
              jnp.asarray(b3), jnp.asarray(w4), jnp.asarray(b4))
    return np.asarray(out).reshape(B, 2, H, W).astype(np.float32)
